# revision 19
# baseline (speedup 1.0000x reference)
"""Trainium2 Bass kernel for DeepGEMM-style masked grouped GEMM (MoE).

Problem (hardcoded shapes):
  E=64 experts, MAX_M=256 tokens/expert, N=1024, K=4096, 128-block dequant
  scales, per-expert valid-token counts masked_m.

Strategy:
  - Expert-parallel over 8 NeuronCores: experts [8c, 8c+8) on core c.
  - Host folds the dequant scales (input_scale per (token, k-block),
    weight_scale per (n-block, k-block)) and the masked_m row mask into the
    operands, casts to bf16, and packs both operands K-major
    ([128 k-partitions, k-tile, free]) so each expert's operands stream to
    SBUF as single large fully-contiguous DMAs.
  - Device: per expert, out[mt] (128xN) = sum over 32 k-tiles of
    aT[kt]^T @ bT[kt] accumulated in PSUM (bf16 matmul, fp32 accumulate),
    then PSUM->SBUF bf16 copy and DMA out. Masked rows are exactly zero
    because the folded mask zeroes those activation rows.
"""

import os

import numpy as np
import ml_dtypes

E, MAX_M, N, K = 64, 256, 1024, 4096
BLK = 128
C = K // BLK  # 32 k-blocks (= k-tiles)
NB = N // BLK  # 8 n-blocks
NCORES = 8
EPC = E // NCORES  # experts per core
NH = 2  # N halves of 512 (one PSUM bank each)
MT = 2  # M tiles of 128

BF16 = ml_dtypes.bfloat16

LAST_EXEC_NS = None


def _build_nc(m_keep):
    """m_keep: number of m-rows shipped/computed per expert (128|192|256).
    Rows >= m_keep are masked-out (zero) for every expert; the output DRAM
    buffer is pre-zeroed by the runtime so untouched rows stay exactly 0.
    """
    import concourse.mybir as mybir
    from concourse import bacc
    from concourse.tile import TileContext

    # m-tiles: (partition_count per tile); mt0 always 128 rows.
    m_tiles = [128] * (m_keep // 128)
    if m_keep % 128:
        m_tiles.append(m_keep % 128)

    nc = bacc.Bacc("TRN2", target_bir_lowering=False, debug=False)
    a_d = nc.dram_tensor(
        "a", [EPC, BLK, C, m_keep], mybir.dt.bfloat16, kind="ExternalInput"
    )
    b_d = nc.dram_tensor(
        "b", [EPC, BLK, C, N], mybir.dt.bfloat16, kind="ExternalInput"
    )
    o_d = nc.dram_tensor(
        "o", [EPC, MT, BLK, N], mybir.dt.bfloat16, kind="ExternalOutput"
    )

    with TileContext(nc) as tc:
        with (
            tc.tile_pool(name="apool", bufs=2) as apool,
            tc.tile_pool(name="bpool", bufs=2) as bpool,
            tc.tile_pool(name="opool", bufs=2) as opool,
            tc.tile_pool(name="psum", bufs=4, space="PSUM") as psum_pool,
        ):
            for i in range(EPC):
                # The walrus DIRECT2D DMA lowering in this toolchain accepts
                # at most ONE sync-wait per DMA instruction. Slot-recycled
                # tiles would put 2 waits (engine WAR + DMA lane) on the
                # load DMA, so a tiny same-engine memset touches the tile
                # first: the memset (a compute op, no wait limit) absorbs
                # the waits and the DMA follows in program order.
                # Early experts' loads are split into chunks (c-ranges):
                # Tile's subtile dependency tracking lets each k-tile's
                # matmuls start as soon as its covering chunk lands, so the
                # pipeline ramps without waiting for whole-expert loads.
                # Later experts use single 8 MiB DMAs (chunking measurably
                # inflates DMA busy time, so only the ramp gets chunks).
                # a goes through the otherwise-idle SP HWDGE queue so it
                # never queues behind an 8 MiB b transfer in the SWDGE ring.
                a_t = apool.tile([BLK, C, m_keep], mybir.dt.bfloat16)
                nc.gpsimd.memset(a_t[0:1, 0, 0:2], 0)
                b_t = bpool.tile([BLK, C, N], mybir.dt.bfloat16)
                nc.gpsimd.memset(b_t[0:1, 0, 0:2], 0)
                if i == 0:
                    b_chunks = [4, 4, 8, 8, 8]
                    nc.sync.dma_start(out=a_t[:, 0:8, :], in_=a_d[i, :, 0:8, :])
                    nc.sync.dma_start(out=a_t[:, 8:C, :], in_=a_d[i, :, 8:C, :])
                elif i == 1:
                    b_chunks = [16, 16]
                    nc.sync.dma_start(out=a_t[:, :, :], in_=a_d[i, :, :, :])
                else:
                    b_chunks = [C]
                    nc.sync.dma_start(out=a_t[:, :, :], in_=a_d[i, :, :, :])
                cg = 0
                for w in b_chunks:
                    nc.gpsimd.dma_start(
                        out=b_t[:, cg : cg + w, :],
                        in_=b_d[i, :, cg : cg + w, :],
                    )
                    cg += w

                o_t = opool.tile([BLK, MT, N], mybir.dt.bfloat16)
                m_off = 0
                for mt, mrows in enumerate(m_tiles):
                    ps = [
                        psum_pool.tile(
                            [BLK, N // NH],
                            mybir.dt.float32,
                            name=f"ps{nh}",
                            tag=f"ps{nh}",
                        )
                        for nh in range(NH)
                    ]
                    for c in range(C):
                        lhsT = a_t[:, c, m_off : m_off + mrows]
                        for nh in range(NH):
                            rhs = b_t[:, c, nh * (N // NH) : (nh + 1) * (N // NH)]
                            nc.tensor.matmul(
                                ps[nh][:mrows, :],
                                lhsT,
                                rhs,
                                start=(c == 0),
                                stop=(c == C - 1),
                            )
                    # PSUM->SBUF cast copies on ACT, and the store DMA issued
                    # from ACT too: the store's RAW dep on the copies is then
                    # same-engine program order (no sem wait on the DMA).
                    for nh in range(NH):
                        nc.scalar.copy(
                            o_t[:mrows, mt, nh * (N // NH) : (nh + 1) * (N // NH)],
                            ps[nh][:mrows, :],
                        )
                    nc.scalar.dma_start(
                        out=o_d[i, mt, 0:mrows, :], in_=o_t[0:mrows, mt, :]
                    )
                    m_off += mrows
    # bacc pass pipeline: moves matmul waits to ldweights and splits
    # over-limit waits into EventSemaphore chains (HW allows 1 wait/inst).
    nc.compile()
    return nc


def _build_nc_paired(m_keep):
    """Expert-PAIR schedule with col-tiled concurrent mt1 matmuls.

    The mt1 tile of each expert only produces m_keep-128 (<=64) output
    partitions, so two experts' mt1 matmuls are placed in distinct PE
    column groups (tile_position (0,0) / (0,64)) and run concurrently,
    cutting PE slots per expert pair from 4 to 3 per (c, nh). `b` streams
    through SBUF in 4-k-tile chunks so two experts' operands plus
    prefetch fit.
    """
    import concourse.mybir as mybir
    from concourse import bacc
    from concourse.tile import TileContext

    assert m_keep in (192, 256)
    m1 = m_keep - 128  # mt1 width (64 when m_keep=192)
    CG = 8  # k-tiles per b chunk (2 MiB chunks keep DMA efficiency high)
    NCH = C // CG  # chunks per expert
    NHW = N // NH  # 512

    nc = bacc.Bacc("TRN2", target_bir_lowering=False, debug=False)
    a_d = nc.dram_tensor(
        "a", [EPC, BLK, C, m_keep], mybir.dt.bfloat16, kind="ExternalInput"
    )
    b_d = nc.dram_tensor(
        "b", [EPC, BLK, C, N], mybir.dt.bfloat16, kind="ExternalInput"
    )
    o_d = nc.dram_tensor(
        "o", [EPC, MT, BLK, N], mybir.dt.bfloat16, kind="ExternalOutput"
    )

    with TileContext(nc) as tc:
        with (
            tc.tile_pool(name="apool", bufs=2) as apool,
            tc.tile_pool(name="bpool", bufs=3) as bpool,
            tc.tile_pool(name="opool", bufs=2) as opool,
            tc.tile_pool(name="psum", bufs=1, space="PSUM") as psum_pool,
        ):
            for p in range(EPC // 2):
                e0, e1 = 2 * p, 2 * p + 1
                ats = []
                for par, e in ((0, e0), (1, e1)):
                    a_t = apool.tile(
                        [BLK, C, m_keep],
                        mybir.dt.bfloat16,
                        name=f"a{par}",
                        tag=f"a{par}",
                    )
                    nc.gpsimd.memset(a_t[0:1, 0, 0:2], 0)
                    nc.gpsimd.dma_start(out=a_t[:, :, :], in_=a_d[e, :, :, :])
                    ats.append(a_t)
                chunks = [[], []]
                for j in range(NCH):
                    for par, e in ((0, e0), (1, e1)):
                        b_t = bpool.tile(
                            [BLK, CG, N],
                            mybir.dt.bfloat16,
                            name=f"b{par}",
                            tag=f"b{par}",
                        )
                        nc.gpsimd.memset(b_t[0:1, 0, 0:2], 0)
                        nc.gpsimd.dma_start(
                            out=b_t[:, :, :],
                            in_=b_d[e, :, j * CG : (j + 1) * CG, :],
                        )
                        chunks[par].append(b_t)

                pm0 = [
                    [
                        psum_pool.tile(
                            [BLK, NHW],
                            mybir.dt.float32,
                            name=f"pm0e{par}n{nh}",
                            tag=f"pm0e{par}n{nh}",
                        )
                        for nh in range(NH)
                    ]
                    for par in range(2)
                ]
                pm1 = [
                    psum_pool.tile(
                        [BLK, NHW],
                        mybir.dt.float32,
                        name=f"pm1n{nh}",
                        tag=f"pm1n{nh}",
                        bufs=2,
                    )
                    for nh in range(NH)
                ]
                for j in range(NCH):
                    for cc in range(CG):
                        c = j * CG + cc
                        st = c == 0
                        sp = c == C - 1
                        for par in range(2):
                            lhsT = ats[par][:, c, 0:128]
                            for nh in range(NH):
                                nc.tensor.matmul(
                                    pm0[par][nh][:, :],
                                    lhsT,
                                    chunks[par][j][:, cc, nh * NHW : (nh + 1) * NHW],
                                    start=st,
                                    stop=sp,
                                )
                        for nh in range(NH):
                            for par, base in ((0, 0), (1, 64)):
                                nc.tensor.matmul(
                                    pm1[nh][base : base + m1, :],
                                    ats[par][:, c, 128 : 128 + m1],
                                    chunks[par][j][:, cc, nh * NHW : (nh + 1) * NHW],
                                    start=st,
                                    stop=sp,
                                    tile_position=(0, base),
                                )
                for par, e in ((0, e0), (1, e1)):
                    o_t = opool.tile(
                        [BLK, MT, N],
                        mybir.dt.bfloat16,
                        name=f"o{par}",
                        tag=f"o{par}",
                    )
                    for nh in range(NH):
                        nc.scalar.copy(
                            o_t[:, 0, nh * NHW : (nh + 1) * NHW],
                            pm0[par][nh][:, :],
                        )
                    nc.scalar.dma_start(out=o_d[e, 0, :, :], in_=o_t[:, 0, :])
                    base = 64 * par
                    for nh in range(NH):
                        nc.scalar.copy(
                            o_t[0:m1, 1, nh * NHW : (nh + 1) * NHW],
                            pm1[nh][base : base + m1, :],
                        )
                    nc.scalar.dma_start(
                        out=o_d[e, 1, 0:m1, :], in_=o_t[0:m1, 1, :]
                    )
    nc.compile()
    return nc


def _ensure_axon_hooks_module():
    """bass_utils' trace path does `from antenv.axon_hooks import ...`;
    this container's antenv lacks that submodule, which would crash
    run_bass_kernel_spmd if BASS_TRACE is set in the environment. Register
    a functional stand-in (ctypes NRT-profile hook) only when missing."""
    import sys

    try:
        import antenv.axon_hooks  # noqa: F401

        return
    except ImportError:
        pass
    import contextlib
    import ctypes
    import types

    mod = types.ModuleType("antenv.axon_hooks")
    state = {"hook": None}
    mod.set_axon_ntff_profile_hook = lambda h: state.__setitem__("hook", h)
    mod.get_axon_ntff_profile_hook = lambda: state["hook"]
    sys.modules["antenv.axon_hooks"] = mod

    try:
        lib = ctypes.CDLL("/opt/axon/libaxon_pjrt.so")
        if not hasattr(lib, "axon_start_nrt_profile"):
            return
        lib.axon_start_nrt_profile.argtypes = [
            ctypes.POINTER(ctypes.c_int64),
            ctypes.c_size_t,
        ]
        lib.axon_start_nrt_profile.restype = ctypes.c_int64
        lib.axon_stop_nrt_profile.argtypes = [ctypes.c_char_p]
        lib.axon_stop_nrt_profile.restype = ctypes.c_int64

        @contextlib.contextmanager
        def _hook(output_dir, device_ids):
            import jax

            jax.devices()
            if device_ids:
                ids = (ctypes.c_int64 * len(device_ids))(*device_ids)
                rc = lib.axon_start_nrt_profile(ids, len(device_ids))
            else:
                rc = lib.axon_start_nrt_profile(None, 0)
            if rc != 0:
                raise RuntimeError(f"axon_start_nrt_profile rc={rc}")
            try:
                yield
            finally:
                lib.axon_stop_nrt_profile(str(output_dir).encode())

        mod.set_axon_ntff_profile_hook(_hook)
    except OSError:
        pass


def kernel(input, input_scale, weight, weight_scale, masked_m):
    global LAST_EXEC_NS
    _ensure_axon_hooks_module()
    from concourse import bass_utils

    inp = np.asarray(input, dtype=np.float32)
    isc = np.asarray(input_scale, dtype=np.float32)
    w = np.asarray(weight, dtype=np.float32)
    wsc = np.asarray(weight_scale, dtype=np.float32)
    mm = np.asarray(masked_m, dtype=np.int32)

    # Rows >= max(masked_m) are masked-out everywhere: don't ship or compute
    # them (their outputs stay zero via the pre-zeroed output buffer).
    mmax = int(mm.max()) if mm.size else 0
    m_keep = min(max(mmax, 128), MAX_M)

    # Fold row mask into the per-token scales: masked rows of `a` become
    # exactly zero, so those output rows are exactly zero after the GEMM.
    mask = (np.arange(m_keep, dtype=np.int32)[None, :] < mm[:, None]).astype(
        np.float32
    )
    # a[e, m, k] = inp * isc[e, m, k//128] * mask  -> bf16
    a = (
        inp[:, :m_keep].reshape(E, m_keep, C, BLK)
        * (isc[:, :m_keep] * mask[:, :, None])[..., None]
    ).astype(BF16)
    # pack K-major: a_packed[e, p, c, m] = a[e, m, c, p]
    a_packed = np.ascontiguousarray(a.transpose(0, 3, 2, 1))

    # b[e, n, k] = w * wsc[e, n//128, k//128]  -> bf16
    b = (w.reshape(E, NB, BLK, C, BLK) * wsc[:, :, None, :, None]).astype(BF16)
    # dims [E, nb, ni, c, p] -> b_packed[e, p, c, nb, ni] -> [E, p, c, N]
    b_packed = np.ascontiguousarray(b.transpose(0, 4, 3, 1, 2)).reshape(
        E, BLK, C, N
    )

    variant = os.environ.get("BASS_KERNEL_VARIANT", "v2")
    if variant == "v4" and m_keep == 192:
        nc = _build_nc_paired(m_keep)
    else:
        nc = _build_nc(m_keep)

    in_maps = [
        {
            "a": a_packed[core * EPC : (core + 1) * EPC],
            "b": b_packed[core * EPC : (core + 1) * EPC],
        }
        for core in range(NCORES)
    ]

    trace = os.environ.get("BASS_KERNEL_TRACE", "") == "1"
    res = bass_utils.run_bass_kernel_spmd(
        nc, in_maps, core_ids=list(range(NCORES)), trace=trace
    )
    LAST_EXEC_NS = res.exec_time_ns

    # o[i, mt, p, n] per core; m = mt*128 + p
    out = np.concatenate([r["o"] for r in res.results], axis=0)  # [E, MT, BLK, N]
    return out.reshape(E, MAX_M, N)
